# revision 11
# baseline (speedup 1.0000x reference)
"""GroupedExperts MoE kernel for Trainium2 (8 NeuronCores, expert-parallel).

Reference computation (per expert e):
    h   = x[e] @ W1[e] + b1[e]              # [T, 2D]
    glu = min(h[..., ::2], 7)
    lin = clip(h[..., 1::2], -7, 7)
    s   = glu * sigmoid(1.702 * glu) * (lin + 1)
    out = s @ W2[e] + b2[e]                 # [T, D]

Shapes: E=8, T=2048, D=2048.  One expert per NeuronCore, no cross-core comm.

Device dataflow is fully transposed (features on partitions, tokens on the
free dim) so no on-chip transposes are needed:
    MM1:  hT[f_chunk] = sum_k W1[k, f_chunk].T @ xT[k]     (W1 stationary)
    MM2:  outT[d_chunk] = sum_f W2[f, d_chunk].T @ sT[f]   (W2 stationary)
W1 is de-interleaved on the host into glu/lin halves, so SwiGLU becomes
elementwise between two separate PSUM drains.

All matmul operands are bf16 (rel err ~4e-3, tolerance 2e-2): same PE rate
as float32r (1 col/cycle) but FWL halves LDWEIGHTS, DMA bytes halve, and
the whole T=2048 token range is processed in ONE pass (weights streamed
once).  Per stationary load the PE now runs 4x512-col matmuls.
PSUM: 4 banks per chunk, glu/lin (and consecutive d) chunks double-buffer.
"""

import os
import sys

import numpy as np
import ml_dtypes

for _p in ("/opt/trn_rl_repo", "/root/.axon_site/_ro/trn_rl_repo"):
    if os.path.isdir(_p) and _p not in sys.path:
        sys.path.append(_p)

import concourse.bass as bass  # noqa: E402
import concourse.mybir as mybir  # noqa: E402
import concourse.tile as tile  # noqa: E402
from concourse import bacc  # noqa: E402
from concourse.bass_utils import run_bass_kernel_spmd  # noqa: E402

E = 8
T = 2048
D = 2048
P = 128
KO = D // P      # 16 k-chunks (contraction over D)
FO = D // P      # 16 feature chunks per glu/lin half
DO = D // P      # 16 output-feature chunks
NSUB = T // 512  # 4 psum sub-tiles across the full token range

ALPHA = 1.702
LIMIT = 7.0

TRACE = False          # test.py sets True to capture an NTFF profile
LAST_RESULTS = None    # test.py reads exec_time_ns / trace path from here

_CACHE = {}

f32 = mybir.dt.float32
bf16 = mybir.dt.bfloat16


def _emit(tc, xt, w1g, w1l, w2p, bias, outT):
    from contextlib import ExitStack

    ctx = ExitStack()
    nc = tc.nc
    Silu = mybir.ActivationFunctionType.Silu
    Ident = mybir.ActivationFunctionType.Identity
    add = mybir.AluOpType.add
    amin = mybir.AluOpType.min
    amax = mybir.AluOpType.max

    const_pool = ctx.enter_context(tc.tile_pool(name="const", bufs=1))
    x_pool = ctx.enter_context(tc.tile_pool(name="xp", bufs=KO))
    s_pool = ctx.enter_context(tc.tile_pool(name="sp", bufs=FO))
    w_pool = ctx.enter_context(tc.tile_pool(name="wp", bufs=6))
    t_pool = ctx.enter_context(tc.tile_pool(name="tp", bufs=4))
    o_pool = ctx.enter_context(tc.tile_pool(name="op", bufs=2))
    ps_pool = ctx.enter_context(tc.tile_pool(name="ps", bufs=8, space="PSUM"))

    bias_sb = const_pool.tile([P, 3 * FO], f32, name="bias_sb")
    b1g_sb = bias_sb[:, 0:FO]
    b1l_sb = bias_sb[:, FO : 2 * FO]
    b2_sb = bias_sb[:, 2 * FO : 3 * FO]

    # ---- PE warm-up: a few matmuls on memset scratch right after the
    # prologue start the clock-ramp window before real data lands.
    scr = const_pool.tile([P, 256], bf16, name="scr")
    nc.gpsimd.memset(scr[:], 0)
    scr_ps = ps_pool.tile([P, 256], f32, tag="ps", name="scr_ps")
    for _ in range(4):
        nc.tensor.matmul(scr_ps[:], scr[:, :P], scr[:], start=True, stop=True)

    # ---- DMA head.  Issuing a [128, N] DMA costs ~0.6us of queue occupancy
    # (128 descriptors) regardless of N, so use FEW, WHOLE-TILE transfers --
    # except the very first operands, which are sliced fine so the first
    # matmul issues ~3us earlier.  x panel + first w-slice ride the sync
    # queue; the rest of the weights ride the scalar (Activation) HWDGE
    # queue so they never queue behind the panel.
    wg0 = w_pool.tile([P, KO, P], bf16, tag="w", name="wg_0")
    wl0 = w_pool.tile([P, KO, P], bf16, tag="w", name="wl_0")
    xts = [x_pool.tile([P, T], bf16, tag="xt", name=f"xt_{k}") for k in range(KO)]
    nc.sync.dma_start(wg0[:, 0, :], w1g[0, :, 0, :])
    nc.scalar.dma_start(wl0[:, 0, :], w1l[0, :, 0, :])
    for c in range(0, T, 512):
        nc.sync.dma_start(xts[0][:, c : c + 512], xt[0, :, c : c + 512])
    nc.scalar.dma_start(wg0[:, 1:, :], w1g[0, :, 1:, :])
    nc.scalar.dma_start(wl0[:, 1:, :], w1l[0, :, 1:, :])
    nc.scalar.dma_start(bias_sb[:], bias)
    for k in range(1, KO):
        nc.sync.dma_start(xts[k][:], xt[k])

    s_tiles = [
        s_pool.tile([P, T], bf16, tag="s", name=f"s_{f}") for f in range(FO)
    ]

    def swiglu_glu(pg, f):
        # s[f] = Silu(ALPHA * min(pg + b1g, LIMIT))  (per 512-col subtile)
        for ns in range(NSUB):
            tg = t_pool.tile([P, 512], f32, tag="t", name=f"tg_{f}_{ns}")
            nc.vector.tensor_scalar(
                out=tg[:],
                in0=pg[ns][:],
                scalar1=b1g_sb[:, f : f + 1],
                scalar2=LIMIT,
                op0=add,
                op1=amin,
            )
            nc.scalar.activation(
                out=s_tiles[f][:, ns * 512 : (ns + 1) * 512],
                in_=tg[:],
                func=Silu,
                scale=ALPHA,
            )

    def swiglu_lin(pl, f):
        # s[f] *= clip((pl + b1l + 1)/ALPHA, (1-LIMIT)/ALPHA, (1+LIMIT)/ALPHA)
        for ns in range(NSUB):
            tl = t_pool.tile([P, 512], f32, tag="t", name=f"tl_{f}_{ns}")
            nc.scalar.activation(
                out=tl[:],
                in_=pl[ns][:],
                func=Ident,
                bias=b1l_sb[:, f : f + 1],
                scale=1.0 / ALPHA,
            )
            nc.vector.tensor_scalar(
                out=tl[:],
                in0=tl[:],
                scalar1=(LIMIT + 1.0) / ALPHA,
                scalar2=(-LIMIT + 1.0) / ALPHA,
                op0=amin,
                op1=amax,
            )
            sl = s_tiles[f][:, ns * 512 : (ns + 1) * 512]
            nc.vector.tensor_mul(out=sl, in0=sl, in1=tl[:])

    # ---- f = 0: glu+lin interleaved per k so each arriving x chunk feeds
    # 8 matmuls (1.7us PE vs 1.6us DMA) -- PE chases the panel fill.
    pg = [ps_pool.tile([P, 512], f32, tag="ps", name=f"pg_0_{ns}") for ns in range(NSUB)]
    pl = [ps_pool.tile([P, 512], f32, tag="ps", name=f"pl_0_{ns}") for ns in range(NSUB)]
    for k in range(KO):
        for part, w in ((pg, wg0), (pl, wl0)):
            for ns in range(NSUB):
                nc.tensor.matmul(
                    part[ns][:],
                    w[:, k, :],
                    xts[k][:, ns * 512 : (ns + 1) * 512],
                    start=(k == 0),
                    stop=(k == KO - 1),
                )
    swiglu_glu(pg, 0)
    swiglu_lin(pl, 0)

    # ---- f >= 1: separate glu / lin chunk passes, 4 PSUM banks each,
    # so one chunk drains while the next computes.
    for f in range(1, FO):
        for half, src in (("g", w1g), ("l", w1l)):
            wt = w_pool.tile([P, KO, P], bf16, tag="w", name=f"w{half}_{f}")
            nc.scalar.dma_start(wt[:], src[f])
            ps = [
                ps_pool.tile([P, 512], f32, tag="ps", name=f"p{half}_{f}_{ns}")
                for ns in range(NSUB)
            ]
            for k in range(KO):
                for ns in range(NSUB):
                    nc.tensor.matmul(
                        ps[ns][:],
                        wt[:, k, :],
                        xts[k][:, ns * 512 : (ns + 1) * 512],
                        start=(k == 0),
                        stop=(k == KO - 1),
                    )
            if half == "g":
                swiglu_glu(ps, f)
            else:
                swiglu_lin(ps, f)

    # ---- MM2 + bias: outT[d] = sum_f W2[f, d].T @ sT[f] + b2[d]
    # Output is written bf16 (rounding ~0.2% << 2e-2 budget): halves the
    # out DMA bytes.  The last d runs ns-outer / f-inner so its drain
    # (acts + slice DMAs) overlaps its own matmul stream instead of
    # serializing after the final matmul.
    for d in range(DO):
        w2t = w_pool.tile([P, FO, P], bf16, tag="w", name=f"w2_{d}")
        nc.scalar.dma_start(w2t[:], w2p[d])
        po = [
            ps_pool.tile([P, 512], f32, tag="ps", name=f"po_{d}_{ns}")
            for ns in range(NSUB)
        ]
        last = d == DO - 1
        ot = o_pool.tile([P, T], bf16, tag="o", name=f"ot_{d}")
        if last:
            for ns in range(NSUB):
                for f in range(FO):
                    nc.tensor.matmul(
                        po[ns][:],
                        w2t[:, f, :],
                        s_tiles[f][:, ns * 512 : (ns + 1) * 512],
                        start=(f == 0),
                        stop=(f == FO - 1),
                    )
                osl = ot[:, ns * 512 : (ns + 1) * 512]
                nc.scalar.activation(
                    out=osl, in_=po[ns][:], func=Ident, bias=b2_sb[:, d : d + 1]
                )
                nc.sync.dma_start(outT[d, :, ns * 512 : (ns + 1) * 512], osl)
        else:
            for f in range(FO):
                for ns in range(NSUB):
                    nc.tensor.matmul(
                        po[ns][:],
                        w2t[:, f, :],
                        s_tiles[f][:, ns * 512 : (ns + 1) * 512],
                        start=(f == 0),
                        stop=(f == FO - 1),
                    )
            for ns in range(NSUB):
                nc.scalar.activation(
                    out=ot[:, ns * 512 : (ns + 1) * 512],
                    in_=po[ns][:],
                    func=Ident,
                    bias=b2_sb[:, d : d + 1],
                )
            nc.sync.dma_start(outT[d], ot[:])

    ctx.close()


def _build():
    if "nc" in _CACHE:
        return _CACHE["nc"]
    nc = bacc.Bacc(
        "TRN2",
        target_bir_lowering=False,
        debug=False,
        enable_asserts=False,
        num_devices=E,
    )
    xt = nc.dram_tensor("xt", (KO, P, T), bf16, kind="ExternalInput").ap()
    w1g = nc.dram_tensor("w1g", (FO, P, KO, P), bf16, kind="ExternalInput").ap()
    w1l = nc.dram_tensor("w1l", (FO, P, KO, P), bf16, kind="ExternalInput").ap()
    w2p = nc.dram_tensor("w2p", (DO, P, FO, P), bf16, kind="ExternalInput").ap()
    bias = nc.dram_tensor("bias", (P, 3 * FO), f32, kind="ExternalInput").ap()
    outT = nc.dram_tensor("outT", (DO, P, T), bf16, kind="ExternalOutput").ap()
    with tile.TileContext(nc) as tc:
        _emit(tc, xt, w1g, w1l, w2p, bias, outT)
    nc.compile()
    _CACHE["nc"] = nc
    return nc


def _pack_w(w):
    # [K, F] -> [fo, p, ko, m] with K = ko*128 + p, F = fo*128 + m
    return np.ascontiguousarray(
        w.reshape(KO, P, FO, P).transpose(2, 1, 0, 3).astype(ml_dtypes.bfloat16)
    )


def _pack_b(b):
    # [F] -> [p, fo]
    return np.ascontiguousarray(b.reshape(FO, P).T)


def kernel(x, mlp1_weight, mlp1_bias, mlp2_weight, mlp2_bias):
    global LAST_RESULTS
    x = np.asarray(x, np.float32)
    mlp1_weight = np.asarray(mlp1_weight, np.float32)
    mlp1_bias = np.asarray(mlp1_bias, np.float32)
    mlp2_weight = np.asarray(mlp2_weight, np.float32)
    mlp2_bias = np.asarray(mlp2_bias, np.float32)

    nc = _build()
    in_maps = []
    for e in range(E):
        w1 = mlp1_weight[e].reshape(D, 2 * D // 2, 2)  # [K, F, 2] even/odd
        b1 = mlp1_bias[e].reshape(D, 2)
        bias_pack = np.concatenate(
            [
                _pack_b(np.ascontiguousarray(b1[:, 0])),
                _pack_b((np.ascontiguousarray(b1[:, 1]) + 1.0) / ALPHA),
                _pack_b(mlp2_bias[e]),
            ],
            axis=1,
        )
        in_maps.append(
            {
                "xt": np.ascontiguousarray(
                    x[e].T.astype(ml_dtypes.bfloat16)
                ).reshape(KO, P, T),
                "w1g": _pack_w(np.ascontiguousarray(w1[:, :, 0])),
                "w1l": _pack_w(np.ascontiguousarray(w1[:, :, 1])),
                "w2p": _pack_w(mlp2_weight[e]),
                "bias": np.ascontiguousarray(bias_pack),
            }
        )

    res = run_bass_kernel_spmd(
        nc, in_maps, core_ids=list(range(E)), trace=TRACE
    )
    LAST_RESULTS = res
    out = np.stack(
        [
            res.results[e]["outT"].astype(np.float32).reshape(D, T).T
            for e in range(E)
        ]
    )
    return np.ascontiguousarray(out)


# revision 18
# speedup vs baseline: 1.0038x; 1.0038x over previous
"""GroupedExperts MoE kernel for Trainium2 (8 NeuronCores, expert-parallel).

Reference computation (per expert e):
    h   = x[e] @ W1[e] + b1[e]              # [T, 2D]
    glu = min(h[..., ::2], 7)
    lin = clip(h[..., 1::2], -7, 7)
    s   = glu * sigmoid(1.702 * glu) * (lin + 1)
    out = s @ W2[e] + b2[e]                 # [T, D]

Shapes: E=8, T=2048, D=2048.  One expert per NeuronCore, no cross-core comm.

Device dataflow is fully transposed (features on partitions, tokens on the
free dim) so no on-chip transposes are needed:
    MM1:  hT[f_chunk] = sum_k W1[k, f_chunk].T @ xT[k]     (W1 stationary)
    MM2:  outT[d_chunk] = sum_f W2[f, d_chunk].T @ sT[f]   (W2 stationary)
W1 is de-interleaved on the host into glu/lin halves, so SwiGLU becomes
elementwise between two separate PSUM drains.

All matmul operands are bf16 (rel err ~4e-3, tolerance 2e-2): same PE rate
as float32r (1 col/cycle) but FWL halves LDWEIGHTS, DMA bytes halve, and
the whole T=2048 token range is processed in ONE pass (weights streamed
once).  Per stationary load the PE now runs 4x512-col matmuls.
PSUM: 4 banks per chunk, glu/lin (and consecutive d) chunks double-buffer.
"""

import os
import sys

import numpy as np
import ml_dtypes

for _p in ("/opt/trn_rl_repo", "/root/.axon_site/_ro/trn_rl_repo"):
    if os.path.isdir(_p) and _p not in sys.path:
        sys.path.append(_p)

import concourse.bass as bass  # noqa: E402
import concourse.mybir as mybir  # noqa: E402
import concourse.tile as tile  # noqa: E402
from concourse import bacc  # noqa: E402
from concourse.bass_utils import run_bass_kernel_spmd  # noqa: E402

E = 8
T = 2048
D = 2048
P = 128
KO = D // P      # 16 k-chunks (contraction over D)
FO = D // P      # 16 feature chunks per glu/lin half
DO = D // P      # 16 output-feature chunks
NSUB = T // 512  # 4 psum sub-tiles across the full token range

ALPHA = 1.702
LIMIT = 7.0

TRACE = False          # test.py sets True to capture an NTFF profile
LAST_RESULTS = None    # test.py reads exec_time_ns / trace path from here

_CACHE = {}

f32 = mybir.dt.float32
bf16 = mybir.dt.bfloat16


def _emit(tc, xt, w1g, w1l, w1h, w2p, bias, outT):
    from contextlib import ExitStack

    ctx = ExitStack()
    nc = tc.nc
    Silu = mybir.ActivationFunctionType.Silu
    Ident = mybir.ActivationFunctionType.Identity
    add = mybir.AluOpType.add
    amin = mybir.AluOpType.min
    amax = mybir.AluOpType.max

    const_pool = ctx.enter_context(tc.tile_pool(name="const", bufs=1))
    x_pool = ctx.enter_context(tc.tile_pool(name="xp", bufs=KO))
    s_pool = ctx.enter_context(tc.tile_pool(name="sp", bufs=FO))
    w_pool = ctx.enter_context(tc.tile_pool(name="wp", bufs=6))
    t_pool = ctx.enter_context(tc.tile_pool(name="tp", bufs=4))
    o_pool = ctx.enter_context(tc.tile_pool(name="op", bufs=2))
    ps_pool = ctx.enter_context(tc.tile_pool(name="ps", bufs=8, space="PSUM"))

    bias_sb = const_pool.tile([P, 3 * FO], f32, name="bias_sb")
    b1g_sb = bias_sb[:, 0:FO]
    b1l_sb = bias_sb[:, FO : 2 * FO]
    b2_sb = bias_sb[:, 2 * FO : 3 * FO]

    # ---- PE warm-up: matmuls on memset scratch right after the prologue
    # start the clock-ramp window ~2us before real data lands.
    scr = const_pool.tile([P, 256], bf16, name="scr")
    nc.vector.memset(scr[:], 0)
    scr_ps = ps_pool.tile([P, 256], f32, tag="ps", name="scr_ps")
    for _ in range(8):
        nc.tensor.matmul(scr_ps[:], scr[:, :P], scr[:], start=True, stop=True)

    # ---- DMA head.  All transfers are whole-tile and dram-contiguous
    # (sub-tile strided DMAs explode descriptor count: ~0.6us issue per 128
    # descriptors).  Two HWDGE queues: weights on sync, x panel on scalar.
    # The first matmuls are fed by small dedicated head tiles (wg0a/wl0a
    # from the host-packed contiguous w1h tensor, x0h from xt[0]'s first
    # half) so the PE starts ~2.5us before the first full 512KB transfers
    # complete; full wg0/wl0/xts[0] duplicate those bytes harmlessly.
    wg0a = const_pool.tile([P, 4, P], bf16, name="wg0a")
    wl0a = const_pool.tile([P, 4, P], bf16, name="wl0a")
    x0h = const_pool.tile([P, 1024], bf16, name="x0h")
    wg0 = w_pool.tile([P, KO, P], bf16, tag="w", name="wg_0")
    wl0 = w_pool.tile([P, KO, P], bf16, tag="w", name="wl_0")
    xts = [x_pool.tile([P, T], bf16, tag="xt", name=f"xt_{k}") for k in range(KO)]
    nc.scalar.dma_start(x0h[:], xt[0, :, 0:1024])
    nc.sync.dma_start(wg0a[:], w1h[0])
    nc.sync.dma_start(wl0a[:], w1h[1])
    nc.sync.dma_start(wg0[:], w1g[0])
    nc.sync.dma_start(wl0[:], w1l[0])
    for k in range(KO):
        nc.scalar.dma_start(xts[k][:], xt[k])
    nc.scalar.dma_start(bias_sb[:], bias)

    s_tiles = [
        s_pool.tile([P, T], bf16, tag="s", name=f"s_{f}") for f in range(FO)
    ]

    def swiglu_glu(pg, f):
        # s[f] = Silu(ALPHA * min(pg + b1g, LIMIT))  (per 512-col subtile)
        for ns in range(NSUB):
            tg = t_pool.tile([P, 512], f32, tag="t", name=f"tg_{f}_{ns}")
            nc.vector.tensor_scalar(
                out=tg[:],
                in0=pg[ns][:],
                scalar1=b1g_sb[:, f : f + 1],
                scalar2=LIMIT,
                op0=add,
                op1=amin,
            )
            nc.scalar.activation(
                out=s_tiles[f][:, ns * 512 : (ns + 1) * 512],
                in_=tg[:],
                func=Silu,
                scale=ALPHA,
            )

    def swiglu_lin(pl, f):
        # s[f] *= clip((pl + b1l + 1)/ALPHA, (1-LIMIT)/ALPHA, (1+LIMIT)/ALPHA)
        for ns in range(NSUB):
            tl = t_pool.tile([P, 512], f32, tag="t", name=f"tl_{f}_{ns}")
            nc.scalar.activation(
                out=tl[:],
                in_=pl[ns][:],
                func=Ident,
                bias=b1l_sb[:, f : f + 1],
                scale=1.0 / ALPHA,
            )
            nc.vector.tensor_scalar(
                out=tl[:],
                in0=tl[:],
                scalar1=(LIMIT + 1.0) / ALPHA,
                scalar2=(-LIMIT + 1.0) / ALPHA,
                op0=amin,
                op1=amax,
            )
            sl = s_tiles[f][:, ns * 512 : (ns + 1) * 512]
            nc.vector.tensor_mul(out=sl, in0=sl, in1=tl[:])

    # ---- f = 0: glu+lin interleaved per k so each arriving x chunk feeds
    # 8 matmuls (1.7us PE vs 1.6us DMA) -- PE chases the panel fill.
    pg = [ps_pool.tile([P, 512], f32, tag="ps", name=f"pg_0_{ns}") for ns in range(NSUB)]
    pl = [ps_pool.tile([P, 512], f32, tag="ps", name=f"pl_0_{ns}") for ns in range(NSUB)]
    for k in range(KO):
        for part, w, wa in ((pg, wg0, wg0a), (pl, wl0, wl0a)):
            lhs = wa[:, k, :] if k < 4 else w[:, k, :]
            for ns in range(NSUB):
                if k == 0 and ns < 2:
                    rhs = x0h[:, ns * 512 : (ns + 1) * 512]
                else:
                    rhs = xts[k][:, ns * 512 : (ns + 1) * 512]
                nc.tensor.matmul(
                    part[ns][:],
                    lhs,
                    rhs,
                    start=(k == 0),
                    stop=(k == KO - 1),
                )
    swiglu_glu(pg, 0)
    swiglu_lin(pl, 0)

    # ---- f >= 1: separate glu / lin chunk passes, 4 PSUM banks each,
    # so one chunk drains while the next computes.
    for f in range(1, FO):
        for half, src in (("g", w1g), ("l", w1l)):
            wt = w_pool.tile([P, KO, P], bf16, tag="w", name=f"w{half}_{f}")
            nc.sync.dma_start(wt[:], src[f])
            ps = [
                ps_pool.tile([P, 512], f32, tag="ps", name=f"p{half}_{f}_{ns}")
                for ns in range(NSUB)
            ]
            for k in range(KO):
                for ns in range(NSUB):
                    nc.tensor.matmul(
                        ps[ns][:],
                        wt[:, k, :],
                        xts[k][:, ns * 512 : (ns + 1) * 512],
                        start=(k == 0),
                        stop=(k == KO - 1),
                    )
            if half == "g":
                swiglu_glu(ps, f)
            else:
                swiglu_lin(ps, f)

    # ---- MM2 + bias: outT[d] = sum_f W2[f, d].T @ sT[f] + b2[d]
    # Output is written bf16 (rounding ~0.2% << 2e-2 budget): halves the
    # out DMA bytes.  The last d runs ns-outer / f-inner so its drain
    # (acts + slice DMAs) overlaps its own matmul stream instead of
    # serializing after the final matmul.
    for d in range(DO):
        w2t = w_pool.tile([P, FO, P], bf16, tag="w", name=f"w2_{d}")
        nc.sync.dma_start(w2t[:], w2p[d])
        po = [
            ps_pool.tile([P, 512], f32, tag="ps", name=f"po_{d}_{ns}")
            for ns in range(NSUB)
        ]
        last = d == DO - 1
        ot = o_pool.tile([P, T], bf16, tag="o", name=f"ot_{d}")
        if last:
            for ns in range(NSUB):
                for f in range(FO):
                    nc.tensor.matmul(
                        po[ns][:],
                        w2t[:, f, :],
                        s_tiles[f][:, ns * 512 : (ns + 1) * 512],
                        start=(f == 0),
                        stop=(f == FO - 1),
                    )
                osl = ot[:, ns * 512 : (ns + 1) * 512]
                nc.scalar.activation(
                    out=osl, in_=po[ns][:], func=Ident, bias=b2_sb[:, d : d + 1]
                )
                nc.sync.dma_start(outT[d, :, ns * 512 : (ns + 1) * 512], osl)
        else:
            for f in range(FO):
                for ns in range(NSUB):
                    nc.tensor.matmul(
                        po[ns][:],
                        w2t[:, f, :],
                        s_tiles[f][:, ns * 512 : (ns + 1) * 512],
                        start=(f == 0),
                        stop=(f == FO - 1),
                    )
            for ns in range(NSUB):
                nc.scalar.activation(
                    out=ot[:, ns * 512 : (ns + 1) * 512],
                    in_=po[ns][:],
                    func=Ident,
                    bias=b2_sb[:, d : d + 1],
                )
            nc.sync.dma_start(outT[d], ot[:])

    ctx.close()


def _build():
    if "nc" in _CACHE:
        return _CACHE["nc"]
    nc = bacc.Bacc(
        "TRN2",
        target_bir_lowering=False,
        debug=False,
        enable_asserts=False,
        num_devices=E,
    )
    xt = nc.dram_tensor("xt", (KO, P, T), bf16, kind="ExternalInput").ap()
    w1g = nc.dram_tensor("w1g", (FO, P, KO, P), bf16, kind="ExternalInput").ap()
    w1l = nc.dram_tensor("w1l", (FO, P, KO, P), bf16, kind="ExternalInput").ap()
    w1h = nc.dram_tensor("w1h", (2, P, 4, P), bf16, kind="ExternalInput").ap()
    w2p = nc.dram_tensor("w2p", (DO, P, FO, P), bf16, kind="ExternalInput").ap()
    bias = nc.dram_tensor("bias", (P, 3 * FO), f32, kind="ExternalInput").ap()
    outT = nc.dram_tensor("outT", (DO, P, T), bf16, kind="ExternalOutput").ap()
    with tile.TileContext(nc) as tc:
        _emit(tc, xt, w1g, w1l, w1h, w2p, bias, outT)
    nc.compile()
    _CACHE["nc"] = nc
    return nc


def _pack_w(w):
    # [K, F] -> [fo, p, ko, m] with K = ko*128 + p, F = fo*128 + m
    return np.ascontiguousarray(
        w.reshape(KO, P, FO, P).transpose(2, 1, 0, 3).astype(ml_dtypes.bfloat16)
    )


def _pack_b(b):
    # [F] -> [p, fo]
    return np.ascontiguousarray(b.reshape(FO, P).T)


def kernel(x, mlp1_weight, mlp1_bias, mlp2_weight, mlp2_bias):
    global LAST_RESULTS
    x = np.asarray(x, np.float32)
    mlp1_weight = np.asarray(mlp1_weight, np.float32)
    mlp1_bias = np.asarray(mlp1_bias, np.float32)
    mlp2_weight = np.asarray(mlp2_weight, np.float32)
    mlp2_bias = np.asarray(mlp2_bias, np.float32)

    nc = _build()
    in_maps = []
    for e in range(E):
        w1 = mlp1_weight[e].reshape(D, 2 * D // 2, 2)  # [K, F, 2] even/odd
        b1 = mlp1_bias[e].reshape(D, 2)
        bias_pack = np.concatenate(
            [
                _pack_b(np.ascontiguousarray(b1[:, 0])),
                _pack_b((np.ascontiguousarray(b1[:, 1]) + 1.0) / ALPHA),
                _pack_b(mlp2_bias[e]),
            ],
            axis=1,
        )
        w1g_arr = _pack_w(np.ascontiguousarray(w1[:, :, 0]))
        w1l_arr = _pack_w(np.ascontiguousarray(w1[:, :, 1]))
        in_maps.append(
            {
                "xt": np.ascontiguousarray(
                    x[e].T.astype(ml_dtypes.bfloat16)
                ).reshape(KO, P, T),
                "w1g": w1g_arr,
                "w1l": w1l_arr,
                "w1h": np.ascontiguousarray(
                    np.stack([w1g_arr[0][:, 0:4, :], w1l_arr[0][:, 0:4, :]])
                ),
                "w2p": _pack_w(mlp2_weight[e]),
                "bias": np.ascontiguousarray(bias_pack),
            }
        )

    res = run_bass_kernel_spmd(
        nc, in_maps, core_ids=list(range(E)), trace=TRACE
    )
    LAST_RESULTS = res
    out = np.stack(
        [
            res.results[e]["outT"].astype(np.float32).reshape(D, T).T
            for e in range(E)
        ]
    )
    return np.ascontiguousarray(out)


# revision 19
# speedup vs baseline: 1.0127x; 1.0089x over previous
"""GroupedExperts MoE kernel for Trainium2 (8 NeuronCores, expert-parallel).

Reference computation (per expert e):
    h   = x[e] @ W1[e] + b1[e]              # [T, 2D]
    glu = min(h[..., ::2], 7)
    lin = clip(h[..., 1::2], -7, 7)
    s   = glu * sigmoid(1.702 * glu) * (lin + 1)
    out = s @ W2[e] + b2[e]                 # [T, D]

Shapes: E=8, T=2048, D=2048.  One expert per NeuronCore, no cross-core comm.

Device dataflow is fully transposed (features on partitions, tokens on the
free dim) so no on-chip transposes are needed:
    MM1:  hT[f_chunk] = sum_k W1[k, f_chunk].T @ xT[k]     (W1 stationary)
    MM2:  outT[d_chunk] = sum_f W2[f, d_chunk].T @ sT[f]   (W2 stationary)
W1 is de-interleaved on the host into glu/lin halves, so SwiGLU becomes
elementwise between two separate PSUM drains.

All matmul operands are bf16 (rel err ~4e-3, tolerance 2e-2): same PE rate
as float32r (1 col/cycle) but FWL halves LDWEIGHTS, DMA bytes halve, and
the whole T=2048 token range is processed in ONE pass (weights streamed
once).  Per stationary load the PE now runs 4x512-col matmuls.
PSUM: 4 banks per chunk, glu/lin (and consecutive d) chunks double-buffer.
"""

import os
import sys

import numpy as np
import ml_dtypes

for _p in ("/opt/trn_rl_repo", "/root/.axon_site/_ro/trn_rl_repo"):
    if os.path.isdir(_p) and _p not in sys.path:
        sys.path.append(_p)

import concourse.bass as bass  # noqa: E402
import concourse.mybir as mybir  # noqa: E402
import concourse.tile as tile  # noqa: E402
from concourse import bacc  # noqa: E402
from concourse.bass_utils import run_bass_kernel_spmd  # noqa: E402

E = 8
T = 2048
D = 2048
P = 128
KO = D // P      # 16 k-chunks (contraction over D)
FO = D // P      # 16 feature chunks per glu/lin half
DO = D // P      # 16 output-feature chunks
NSUB = T // 512  # 4 psum sub-tiles across the full token range

ALPHA = 1.702
LIMIT = 7.0

TRACE = False          # test.py sets True to capture an NTFF profile
LAST_RESULTS = None    # test.py reads exec_time_ns / trace path from here

_CACHE = {}

f32 = mybir.dt.float32
bf16 = mybir.dt.bfloat16


def _emit(tc, xt, w1g, w1l, w1h, w2p, bias, outT):
    from contextlib import ExitStack

    ctx = ExitStack()
    nc = tc.nc
    Silu = mybir.ActivationFunctionType.Silu
    Ident = mybir.ActivationFunctionType.Identity
    add = mybir.AluOpType.add
    amin = mybir.AluOpType.min
    amax = mybir.AluOpType.max

    const_pool = ctx.enter_context(tc.tile_pool(name="const", bufs=1))
    x_pool = ctx.enter_context(tc.tile_pool(name="xp", bufs=KO))
    s_pool = ctx.enter_context(tc.tile_pool(name="sp", bufs=FO))
    w_pool = ctx.enter_context(tc.tile_pool(name="wp", bufs=6))
    t_pool = ctx.enter_context(tc.tile_pool(name="tp", bufs=4))
    o_pool = ctx.enter_context(tc.tile_pool(name="op", bufs=2))
    ps_pool = ctx.enter_context(tc.tile_pool(name="ps", bufs=8, space="PSUM"))

    bias_sb = const_pool.tile([P, 3 * FO], f32, name="bias_sb")
    b1g_sb = bias_sb[:, 0:FO]
    b1l_sb = bias_sb[:, FO : 2 * FO]
    b2_sb = bias_sb[:, 2 * FO : 3 * FO]

    # ---- PE warm-up: matmuls on memset scratch right after the prologue
    # start the clock-ramp window ~2us before real data lands.
    scr = const_pool.tile([P, 256], bf16, name="scr")
    nc.vector.memset(scr[:], 0)
    scr_ps = ps_pool.tile([P, 256], f32, tag="ps", name="scr_ps")
    for _ in range(8):
        nc.tensor.matmul(scr_ps[:], scr[:, :P], scr[:], start=True, stop=True)

    # ---- DMA head.  All transfers are whole-tile and dram-contiguous
    # (sub-tile strided DMAs explode descriptor count: ~0.6us issue per 128
    # descriptors).  Two HWDGE queues: weights on sync, x panel on scalar.
    # The first matmuls are fed by small dedicated head tiles (wg0a/wl0a
    # from the host-packed contiguous w1h tensor, x0h from xt[0]'s first
    # half) so the PE starts ~2.5us before the first full 512KB transfers
    # complete; full wg0/wl0/xts[0] duplicate those bytes harmlessly.
    wg0a = const_pool.tile([P, 4, P], bf16, name="wg0a")
    wl0a = const_pool.tile([P, 4, P], bf16, name="wl0a")
    x0h = const_pool.tile([P, 1024], bf16, name="x0h")
    wg0 = w_pool.tile([P, KO, P], bf16, tag="w", name="wg_0")
    wl0 = w_pool.tile([P, KO, P], bf16, tag="w", name="wl_0")
    xts = [x_pool.tile([P, T], bf16, tag="xt", name=f"xt_{k}") for k in range(KO)]
    # x panel split even/odd across both queues (aggregate ~400GB/s beats
    # the PE's 1.7us/chunk consumption; one queue alone does not).
    nc.sync.dma_start(x0h[:], xt[0, :, 0:1024])
    nc.scalar.dma_start(xts[0][:], xt[0])
    nc.sync.dma_start(wg0a[:], w1h[0])
    nc.sync.dma_start(wl0a[:], w1h[1])
    nc.sync.dma_start(xts[1][:], xt[1])
    nc.scalar.dma_start(xts[2][:], xt[2])
    nc.sync.dma_start(xts[3][:], xt[3])
    nc.scalar.dma_start(wg0[:], w1g[0])
    nc.sync.dma_start(xts[5][:], xt[5])
    nc.scalar.dma_start(xts[4][:], xt[4])
    nc.sync.dma_start(wl0[:], w1l[0])
    for k in range(6, KO):
        q = nc.sync if k % 2 else nc.scalar
        q.dma_start(xts[k][:], xt[k])
    nc.scalar.dma_start(bias_sb[:], bias)

    s_tiles = [
        s_pool.tile([P, T], bf16, tag="s", name=f"s_{f}") for f in range(FO)
    ]

    def swiglu_glu(pg, f):
        # s[f] = Silu(ALPHA * min(pg + b1g, LIMIT))  (per 512-col subtile)
        for ns in range(NSUB):
            tg = t_pool.tile([P, 512], f32, tag="t", name=f"tg_{f}_{ns}")
            nc.vector.tensor_scalar(
                out=tg[:],
                in0=pg[ns][:],
                scalar1=b1g_sb[:, f : f + 1],
                scalar2=LIMIT,
                op0=add,
                op1=amin,
            )
            nc.scalar.activation(
                out=s_tiles[f][:, ns * 512 : (ns + 1) * 512],
                in_=tg[:],
                func=Silu,
                scale=ALPHA,
            )

    def swiglu_lin(pl, f):
        # s[f] *= clip((pl + b1l + 1)/ALPHA, (1-LIMIT)/ALPHA, (1+LIMIT)/ALPHA)
        for ns in range(NSUB):
            tl = t_pool.tile([P, 512], f32, tag="t", name=f"tl_{f}_{ns}")
            nc.scalar.activation(
                out=tl[:],
                in_=pl[ns][:],
                func=Ident,
                bias=b1l_sb[:, f : f + 1],
                scale=1.0 / ALPHA,
            )
            nc.vector.tensor_scalar(
                out=tl[:],
                in0=tl[:],
                scalar1=(LIMIT + 1.0) / ALPHA,
                scalar2=(-LIMIT + 1.0) / ALPHA,
                op0=amin,
                op1=amax,
            )
            sl = s_tiles[f][:, ns * 512 : (ns + 1) * 512]
            nc.vector.tensor_mul(out=sl, in0=sl, in1=tl[:])

    # ---- f = 0: glu+lin interleaved per k so each arriving x chunk feeds
    # 8 matmuls (1.7us PE vs 1.6us DMA) -- PE chases the panel fill.
    pg = [ps_pool.tile([P, 512], f32, tag="ps", name=f"pg_0_{ns}") for ns in range(NSUB)]
    pl = [ps_pool.tile([P, 512], f32, tag="ps", name=f"pl_0_{ns}") for ns in range(NSUB)]
    for k in range(KO):
        for part, w, wa in ((pg, wg0, wg0a), (pl, wl0, wl0a)):
            lhs = wa[:, k, :] if k < 4 else w[:, k, :]
            for ns in range(NSUB):
                if k == 0 and ns < 2:
                    rhs = x0h[:, ns * 512 : (ns + 1) * 512]
                else:
                    rhs = xts[k][:, ns * 512 : (ns + 1) * 512]
                nc.tensor.matmul(
                    part[ns][:],
                    lhs,
                    rhs,
                    start=(k == 0),
                    stop=(k == KO - 1),
                )
    swiglu_glu(pg, 0)
    swiglu_lin(pl, 0)

    # ---- f >= 1: separate glu / lin chunk passes, 4 PSUM banks each,
    # so one chunk drains while the next computes.
    for f in range(1, FO):
        for half, src in (("g", w1g), ("l", w1l)):
            wt = w_pool.tile([P, KO, P], bf16, tag="w", name=f"w{half}_{f}")
            nc.sync.dma_start(wt[:], src[f])
            ps = [
                ps_pool.tile([P, 512], f32, tag="ps", name=f"p{half}_{f}_{ns}")
                for ns in range(NSUB)
            ]
            for k in range(KO):
                for ns in range(NSUB):
                    nc.tensor.matmul(
                        ps[ns][:],
                        wt[:, k, :],
                        xts[k][:, ns * 512 : (ns + 1) * 512],
                        start=(k == 0),
                        stop=(k == KO - 1),
                    )
            if half == "g":
                swiglu_glu(ps, f)
            else:
                swiglu_lin(ps, f)

    # ---- MM2 + bias: outT[d] = sum_f W2[f, d].T @ sT[f] + b2[d]
    # Output is written bf16 (rounding ~0.2% << 2e-2 budget): halves the
    # out DMA bytes.  The last d runs ns-outer / f-inner so its drain
    # (acts + slice DMAs) overlaps its own matmul stream instead of
    # serializing after the final matmul.
    for d in range(DO):
        w2t = w_pool.tile([P, FO, P], bf16, tag="w", name=f"w2_{d}")
        nc.sync.dma_start(w2t[:], w2p[d])
        po = [
            ps_pool.tile([P, 512], f32, tag="ps", name=f"po_{d}_{ns}")
            for ns in range(NSUB)
        ]
        last = d == DO - 1
        ot = o_pool.tile([P, T], bf16, tag="o", name=f"ot_{d}")
        if last:
            for ns in range(NSUB):
                for f in range(FO):
                    nc.tensor.matmul(
                        po[ns][:],
                        w2t[:, f, :],
                        s_tiles[f][:, ns * 512 : (ns + 1) * 512],
                        start=(f == 0),
                        stop=(f == FO - 1),
                    )
                osl = ot[:, ns * 512 : (ns + 1) * 512]
                nc.scalar.activation(
                    out=osl, in_=po[ns][:], func=Ident, bias=b2_sb[:, d : d + 1]
                )
                nc.sync.dma_start(outT[d, :, ns * 512 : (ns + 1) * 512], osl)
        else:
            for f in range(FO):
                for ns in range(NSUB):
                    nc.tensor.matmul(
                        po[ns][:],
                        w2t[:, f, :],
                        s_tiles[f][:, ns * 512 : (ns + 1) * 512],
                        start=(f == 0),
                        stop=(f == FO - 1),
                    )
            for ns in range(NSUB):
                nc.scalar.activation(
                    out=ot[:, ns * 512 : (ns + 1) * 512],
                    in_=po[ns][:],
                    func=Ident,
                    bias=b2_sb[:, d : d + 1],
                )
            nc.sync.dma_start(outT[d], ot[:])

    ctx.close()


def _build():
    if "nc" in _CACHE:
        return _CACHE["nc"]
    nc = bacc.Bacc(
        "TRN2",
        target_bir_lowering=False,
        debug=False,
        enable_asserts=False,
        num_devices=E,
    )
    xt = nc.dram_tensor("xt", (KO, P, T), bf16, kind="ExternalInput").ap()
    w1g = nc.dram_tensor("w1g", (FO, P, KO, P), bf16, kind="ExternalInput").ap()
    w1l = nc.dram_tensor("w1l", (FO, P, KO, P), bf16, kind="ExternalInput").ap()
    w1h = nc.dram_tensor("w1h", (2, P, 4, P), bf16, kind="ExternalInput").ap()
    w2p = nc.dram_tensor("w2p", (DO, P, FO, P), bf16, kind="ExternalInput").ap()
    bias = nc.dram_tensor("bias", (P, 3 * FO), f32, kind="ExternalInput").ap()
    outT = nc.dram_tensor("outT", (DO, P, T), bf16, kind="ExternalOutput").ap()
    with tile.TileContext(nc) as tc:
        _emit(tc, xt, w1g, w1l, w1h, w2p, bias, outT)
    nc.compile()
    _CACHE["nc"] = nc
    return nc


def _pack_w(w):
    # [K, F] -> [fo, p, ko, m] with K = ko*128 + p, F = fo*128 + m
    return np.ascontiguousarray(
        w.reshape(KO, P, FO, P).transpose(2, 1, 0, 3).astype(ml_dtypes.bfloat16)
    )


def _pack_b(b):
    # [F] -> [p, fo]
    return np.ascontiguousarray(b.reshape(FO, P).T)


def kernel(x, mlp1_weight, mlp1_bias, mlp2_weight, mlp2_bias):
    global LAST_RESULTS
    x = np.asarray(x, np.float32)
    mlp1_weight = np.asarray(mlp1_weight, np.float32)
    mlp1_bias = np.asarray(mlp1_bias, np.float32)
    mlp2_weight = np.asarray(mlp2_weight, np.float32)
    mlp2_bias = np.asarray(mlp2_bias, np.float32)

    nc = _build()
    in_maps = []
    for e in range(E):
        w1 = mlp1_weight[e].reshape(D, 2 * D // 2, 2)  # [K, F, 2] even/odd
        b1 = mlp1_bias[e].reshape(D, 2)
        bias_pack = np.concatenate(
            [
                _pack_b(np.ascontiguousarray(b1[:, 0])),
                _pack_b((np.ascontiguousarray(b1[:, 1]) + 1.0) / ALPHA),
                _pack_b(mlp2_bias[e]),
            ],
            axis=1,
        )
        w1g_arr = _pack_w(np.ascontiguousarray(w1[:, :, 0]))
        w1l_arr = _pack_w(np.ascontiguousarray(w1[:, :, 1]))
        in_maps.append(
            {
                "xt": np.ascontiguousarray(
                    x[e].T.astype(ml_dtypes.bfloat16)
                ).reshape(KO, P, T),
                "w1g": w1g_arr,
                "w1l": w1l_arr,
                "w1h": np.ascontiguousarray(
                    np.stack([w1g_arr[0][:, 0:4, :], w1l_arr[0][:, 0:4, :]])
                ),
                "w2p": _pack_w(mlp2_weight[e]),
                "bias": np.ascontiguousarray(bias_pack),
            }
        )

    res = run_bass_kernel_spmd(
        nc, in_maps, core_ids=list(range(E)), trace=TRACE
    )
    LAST_RESULTS = res
    out = np.stack(
        [
            res.results[e]["outT"].astype(np.float32).reshape(D, T).T
            for e in range(E)
        ]
    )
    return np.ascontiguousarray(out)


# revision 20
# speedup vs baseline: 1.0139x; 1.0012x over previous
"""GroupedExperts MoE kernel for Trainium2 (8 NeuronCores, expert-parallel).

Reference computation (per expert e):
    h   = x[e] @ W1[e] + b1[e]              # [T, 2D]
    glu = min(h[..., ::2], 7)
    lin = clip(h[..., 1::2], -7, 7)
    s   = glu * sigmoid(1.702 * glu) * (lin + 1)
    out = s @ W2[e] + b2[e]                 # [T, D]

Shapes: E=8, T=2048, D=2048.  One expert per NeuronCore, no cross-core comm.

Device dataflow is fully transposed (features on partitions, tokens on the
free dim) so no on-chip transposes are needed:
    MM1:  hT[f_chunk] = sum_k W1[k, f_chunk].T @ xT[k]     (W1 stationary)
    MM2:  outT[d_chunk] = sum_f W2[f, d_chunk].T @ sT[f]   (W2 stationary)
W1 is de-interleaved on the host into glu/lin halves, so SwiGLU becomes
elementwise between two separate PSUM drains.

All matmul operands are bf16 (rel err ~4e-3, tolerance 2e-2): same PE rate
as float32r (1 col/cycle) but FWL halves LDWEIGHTS, DMA bytes halve, and
the whole T=2048 token range is processed in ONE pass (weights streamed
once).  Per stationary load the PE now runs 4x512-col matmuls.
PSUM: 4 banks per chunk, glu/lin (and consecutive d) chunks double-buffer.
"""

import os
import sys

import numpy as np
import ml_dtypes

for _p in ("/opt/trn_rl_repo", "/root/.axon_site/_ro/trn_rl_repo"):
    if os.path.isdir(_p) and _p not in sys.path:
        sys.path.append(_p)

import concourse.bass as bass  # noqa: E402
import concourse.mybir as mybir  # noqa: E402
import concourse.tile as tile  # noqa: E402
from concourse import bacc  # noqa: E402
from concourse.bass_utils import run_bass_kernel_spmd  # noqa: E402

E = 8
T = 2048
D = 2048
P = 128
KO = D // P      # 16 k-chunks (contraction over D)
FO = D // P      # 16 feature chunks per glu/lin half
DO = D // P      # 16 output-feature chunks
NSUB = T // 512  # 4 psum sub-tiles across the full token range

ALPHA = 1.702
LIMIT = 7.0

TRACE = False          # test.py sets True to capture an NTFF profile
LAST_RESULTS = None    # test.py reads exec_time_ns / trace path from here

_CACHE = {}

f32 = mybir.dt.float32
bf16 = mybir.dt.bfloat16


def _emit(tc, xt, w1g, w1l, w1h, w2p, bias, outT):
    from contextlib import ExitStack

    ctx = ExitStack()
    nc = tc.nc
    Silu = mybir.ActivationFunctionType.Silu
    Ident = mybir.ActivationFunctionType.Identity
    add = mybir.AluOpType.add
    amin = mybir.AluOpType.min
    amax = mybir.AluOpType.max

    const_pool = ctx.enter_context(tc.tile_pool(name="const", bufs=1))
    x_pool = ctx.enter_context(tc.tile_pool(name="xp", bufs=KO))
    s_pool = ctx.enter_context(tc.tile_pool(name="sp", bufs=FO))
    w_pool = ctx.enter_context(tc.tile_pool(name="wp", bufs=6))
    t_pool = ctx.enter_context(tc.tile_pool(name="tp", bufs=4))
    o_pool = ctx.enter_context(tc.tile_pool(name="op", bufs=2))
    ps_pool = ctx.enter_context(tc.tile_pool(name="ps", bufs=8, space="PSUM"))

    bias_sb = const_pool.tile([P, 3 * FO], f32, name="bias_sb")
    b1g_sb = bias_sb[:, 0:FO]
    b1l_sb = bias_sb[:, FO : 2 * FO]
    b2_sb = bias_sb[:, 2 * FO : 3 * FO]

    # ---- PE warm-up: matmuls on memset scratch right after the prologue
    # start the clock-ramp window ~2us before real data lands.
    scr = const_pool.tile([P, 256], bf16, name="scr")
    nc.vector.memset(scr[:], 0)
    scr_ps = ps_pool.tile([P, 256], f32, tag="ps", name="scr_ps")
    for _ in range(16):
        nc.tensor.matmul(scr_ps[:], scr[:, :P], scr[:], start=True, stop=True)

    # ---- DMA head.  All transfers are whole-tile and dram-contiguous
    # (sub-tile strided DMAs explode descriptor count: ~0.6us issue per 128
    # descriptors).  Two HWDGE queues: weights on sync, x panel on scalar.
    # The first matmuls are fed by small dedicated head tiles (wg0a/wl0a
    # from the host-packed contiguous w1h tensor, x0h from xt[0]'s first
    # half) so the PE starts ~2.5us before the first full 512KB transfers
    # complete; full wg0/wl0/xts[0] duplicate those bytes harmlessly.
    wg0a = const_pool.tile([P, 4, P], bf16, name="wg0a")
    wl0a = const_pool.tile([P, 4, P], bf16, name="wl0a")
    x0h = const_pool.tile([P, 1024], bf16, name="x0h")
    wg0 = w_pool.tile([P, KO, P], bf16, tag="w", name="wg_0")
    wl0 = w_pool.tile([P, KO, P], bf16, tag="w", name="wl_0")
    xts = [x_pool.tile([P, T], bf16, tag="xt", name=f"xt_{k}") for k in range(KO)]
    # x panel split even/odd across both queues (aggregate ~400GB/s beats
    # the PE's 1.7us/chunk consumption; one queue alone does not).
    nc.sync.dma_start(x0h[:], xt[0, :, 0:1024])
    nc.scalar.dma_start(xts[0][:], xt[0])
    nc.sync.dma_start(wg0a[:], w1h[0])
    nc.sync.dma_start(wl0a[:], w1h[1])
    nc.sync.dma_start(xts[1][:], xt[1])
    nc.scalar.dma_start(xts[2][:], xt[2])
    nc.sync.dma_start(xts[3][:], xt[3])
    nc.scalar.dma_start(wg0[:], w1g[0])
    nc.sync.dma_start(xts[5][:], xt[5])
    nc.scalar.dma_start(xts[4][:], xt[4])
    nc.sync.dma_start(wl0[:], w1l[0])
    for k in range(6, KO):
        q = nc.sync if k % 2 else nc.scalar
        q.dma_start(xts[k][:], xt[k])
    nc.scalar.dma_start(bias_sb[:], bias)

    s_tiles = [
        s_pool.tile([P, T], bf16, tag="s", name=f"s_{f}") for f in range(FO)
    ]

    def swiglu_glu(pg, f):
        # s[f] = Silu(ALPHA * min(pg + b1g, LIMIT))  (per 512-col subtile)
        for ns in range(NSUB):
            tg = t_pool.tile([P, 512], f32, tag="t", name=f"tg_{f}_{ns}")
            nc.vector.tensor_scalar(
                out=tg[:],
                in0=pg[ns][:],
                scalar1=b1g_sb[:, f : f + 1],
                scalar2=LIMIT,
                op0=add,
                op1=amin,
            )
            nc.scalar.activation(
                out=s_tiles[f][:, ns * 512 : (ns + 1) * 512],
                in_=tg[:],
                func=Silu,
                scale=ALPHA,
            )

    def swiglu_lin(pl, f):
        # s[f] *= clip((pl + b1l + 1)/ALPHA, (1-LIMIT)/ALPHA, (1+LIMIT)/ALPHA)
        for ns in range(NSUB):
            tl = t_pool.tile([P, 512], f32, tag="t", name=f"tl_{f}_{ns}")
            nc.scalar.activation(
                out=tl[:],
                in_=pl[ns][:],
                func=Ident,
                bias=b1l_sb[:, f : f + 1],
                scale=1.0 / ALPHA,
            )
            nc.vector.tensor_scalar(
                out=tl[:],
                in0=tl[:],
                scalar1=(LIMIT + 1.0) / ALPHA,
                scalar2=(-LIMIT + 1.0) / ALPHA,
                op0=amin,
                op1=amax,
            )
            sl = s_tiles[f][:, ns * 512 : (ns + 1) * 512]
            nc.vector.tensor_mul(out=sl, in0=sl, in1=tl[:])

    # ---- f = 0: glu+lin interleaved per k so each arriving x chunk feeds
    # 8 matmuls (1.7us PE vs 1.6us DMA) -- PE chases the panel fill.
    pg = [ps_pool.tile([P, 512], f32, tag="ps", name=f"pg_0_{ns}") for ns in range(NSUB)]
    pl = [ps_pool.tile([P, 512], f32, tag="ps", name=f"pl_0_{ns}") for ns in range(NSUB)]
    for k in range(KO):
        for part, w, wa in ((pg, wg0, wg0a), (pl, wl0, wl0a)):
            lhs = wa[:, k, :] if k < 4 else w[:, k, :]
            for ns in range(NSUB):
                if k == 0 and ns < 2:
                    rhs = x0h[:, ns * 512 : (ns + 1) * 512]
                else:
                    rhs = xts[k][:, ns * 512 : (ns + 1) * 512]
                nc.tensor.matmul(
                    part[ns][:],
                    lhs,
                    rhs,
                    start=(k == 0),
                    stop=(k == KO - 1),
                )
    swiglu_glu(pg, 0)
    swiglu_lin(pl, 0)

    # ---- f >= 1: separate glu / lin chunk passes, 4 PSUM banks each,
    # so one chunk drains while the next computes.
    for f in range(1, FO):
        for half, src in (("g", w1g), ("l", w1l)):
            wt = w_pool.tile([P, KO, P], bf16, tag="w", name=f"w{half}_{f}")
            nc.sync.dma_start(wt[:], src[f])
            ps = [
                ps_pool.tile([P, 512], f32, tag="ps", name=f"p{half}_{f}_{ns}")
                for ns in range(NSUB)
            ]
            for k in range(KO):
                for ns in range(NSUB):
                    nc.tensor.matmul(
                        ps[ns][:],
                        wt[:, k, :],
                        xts[k][:, ns * 512 : (ns + 1) * 512],
                        start=(k == 0),
                        stop=(k == KO - 1),
                    )
            if half == "g":
                swiglu_glu(ps, f)
            else:
                swiglu_lin(ps, f)

    # ---- MM2 + bias: outT[d] = sum_f W2[f, d].T @ sT[f] + b2[d]
    # Output is written bf16 (rounding ~0.2% << 2e-2 budget): halves the
    # out DMA bytes.  The last d runs ns-outer / f-inner so its drain
    # (acts + slice DMAs) overlaps its own matmul stream instead of
    # serializing after the final matmul.
    for d in range(DO):
        w2t = w_pool.tile([P, FO, P], bf16, tag="w", name=f"w2_{d}")
        nc.sync.dma_start(w2t[:], w2p[d])
        po = [
            ps_pool.tile([P, 512], f32, tag="ps", name=f"po_{d}_{ns}")
            for ns in range(NSUB)
        ]
        last = d == DO - 1
        ot = o_pool.tile([P, T], bf16, tag="o", name=f"ot_{d}")
        if last:
            for ns in range(NSUB):
                for f in range(FO):
                    nc.tensor.matmul(
                        po[ns][:],
                        w2t[:, f, :],
                        s_tiles[f][:, ns * 512 : (ns + 1) * 512],
                        start=(f == 0),
                        stop=(f == FO - 1),
                    )
                osl = ot[:, ns * 512 : (ns + 1) * 512]
                nc.scalar.activation(
                    out=osl, in_=po[ns][:], func=Ident, bias=b2_sb[:, d : d + 1]
                )
                nc.sync.dma_start(outT[d, :, ns * 512 : (ns + 1) * 512], osl)
        else:
            for f in range(FO):
                for ns in range(NSUB):
                    nc.tensor.matmul(
                        po[ns][:],
                        w2t[:, f, :],
                        s_tiles[f][:, ns * 512 : (ns + 1) * 512],
                        start=(f == 0),
                        stop=(f == FO - 1),
                    )
            for ns in range(NSUB):
                nc.scalar.activation(
                    out=ot[:, ns * 512 : (ns + 1) * 512],
                    in_=po[ns][:],
                    func=Ident,
                    bias=b2_sb[:, d : d + 1],
                )
            nc.sync.dma_start(outT[d], ot[:])

    ctx.close()


def _build():
    if "nc" in _CACHE:
        return _CACHE["nc"]
    nc = bacc.Bacc(
        "TRN2",
        target_bir_lowering=False,
        debug=False,
        enable_asserts=False,
        num_devices=E,
    )
    xt = nc.dram_tensor("xt", (KO, P, T), bf16, kind="ExternalInput").ap()
    w1g = nc.dram_tensor("w1g", (FO, P, KO, P), bf16, kind="ExternalInput").ap()
    w1l = nc.dram_tensor("w1l", (FO, P, KO, P), bf16, kind="ExternalInput").ap()
    w1h = nc.dram_tensor("w1h", (2, P, 4, P), bf16, kind="ExternalInput").ap()
    w2p = nc.dram_tensor("w2p", (DO, P, FO, P), bf16, kind="ExternalInput").ap()
    bias = nc.dram_tensor("bias", (P, 3 * FO), f32, kind="ExternalInput").ap()
    outT = nc.dram_tensor("outT", (DO, P, T), bf16, kind="ExternalOutput").ap()
    with tile.TileContext(nc) as tc:
        _emit(tc, xt, w1g, w1l, w1h, w2p, bias, outT)
    nc.compile()
    _CACHE["nc"] = nc
    return nc


def _pack_w(w):
    # [K, F] -> [fo, p, ko, m] with K = ko*128 + p, F = fo*128 + m
    return np.ascontiguousarray(
        w.reshape(KO, P, FO, P).transpose(2, 1, 0, 3).astype(ml_dtypes.bfloat16)
    )


def _pack_b(b):
    # [F] -> [p, fo]
    return np.ascontiguousarray(b.reshape(FO, P).T)


def kernel(x, mlp1_weight, mlp1_bias, mlp2_weight, mlp2_bias):
    global LAST_RESULTS
    x = np.asarray(x, np.float32)
    mlp1_weight = np.asarray(mlp1_weight, np.float32)
    mlp1_bias = np.asarray(mlp1_bias, np.float32)
    mlp2_weight = np.asarray(mlp2_weight, np.float32)
    mlp2_bias = np.asarray(mlp2_bias, np.float32)

    nc = _build()
    in_maps = []
    for e in range(E):
        w1 = mlp1_weight[e].reshape(D, 2 * D // 2, 2)  # [K, F, 2] even/odd
        b1 = mlp1_bias[e].reshape(D, 2)
        bias_pack = np.concatenate(
            [
                _pack_b(np.ascontiguousarray(b1[:, 0])),
                _pack_b((np.ascontiguousarray(b1[:, 1]) + 1.0) / ALPHA),
                _pack_b(mlp2_bias[e]),
            ],
            axis=1,
        )
        w1g_arr = _pack_w(np.ascontiguousarray(w1[:, :, 0]))
        w1l_arr = _pack_w(np.ascontiguousarray(w1[:, :, 1]))
        in_maps.append(
            {
                "xt": np.ascontiguousarray(
                    x[e].T.astype(ml_dtypes.bfloat16)
                ).reshape(KO, P, T),
                "w1g": w1g_arr,
                "w1l": w1l_arr,
                "w1h": np.ascontiguousarray(
                    np.stack([w1g_arr[0][:, 0:4, :], w1l_arr[0][:, 0:4, :]])
                ),
                "w2p": _pack_w(mlp2_weight[e]),
                "bias": np.ascontiguousarray(bias_pack),
            }
        )

    res = run_bass_kernel_spmd(
        nc, in_maps, core_ids=list(range(E)), trace=TRACE
    )
    LAST_RESULTS = res
    out = np.stack(
        [
            res.results[e]["outT"].astype(np.float32).reshape(D, T).T
            for e in range(E)
        ]
    )
    return np.ascontiguousarray(out)
